# revision 1
# baseline (speedup 1.0000x reference)
"""Laplacian normalization kernel for Trainium2 (8 NeuronCores, SPMD).

out = D^-1/2 A D^-1/2 where D = diag(row sums of A), A: [8192, 8192] fp32.

Sharding: rows split across 8 cores (1024 rows each). Per core:
  pass 1: stream stripes 0-3 first (quarter-width units), then load
    stripes 4-7 into RESIDENT SBUF tiles (16MB cache). Row sums reduce
    per unit; isq = 1/sqrt(deg) is finished per stripe.
  TWO AllGathers: AG1 ships stripes 0-3's isq chunks while stripes 4-7
    are still loading, AG2 ships the rest. AG1's output covers every
    output column j with (j mod 1024) < 512, so half of the scaling and
    stores run during the window where the kernel used to idle waiting
    on a single collective (which is bound by the slowest core).
  pass 2: out = (A * r[:,None]) * c[None,:], one fused DVE op per
    (unit, collective-half), strided over the covered column ranges.

Ring discipline: pass-2 reloads ride the sync HWDGE ring and stores ride
the scalar ring exclusively, so a store blocked on a post-collective
multiply can never sit ahead of an eligible reload in ring FIFO order.
Tiny latency-critical DMAs (isq writes, broadcasts) go via SWDGE.
"""

import sys

sys.path.insert(0, "/opt/trn_rl_repo")

import numpy as np

import concourse.bacc as bacc
import concourse.tile as tile
from concourse import mybir
from concourse.bass_utils import run_bass_kernel_spmd

N = 8192          # full matrix dim
CORES = 8
R = N // CORES    # rows per core: 1024
P = 128           # partitions
S = R // P        # row stripes per core: 8
HW = 4096         # resident half width
QW = 2048         # stream quarter width
NRES = 4          # stripes 4-7 resident in SBUF
HAG = R // 2      # isq elements per collective half: 512
F32 = mybir.dt.float32
MUL = mybir.AluOpType.mult
X = mybir.AxisListType.X

_CACHE = {}


def build_nc():
    if "nc" in _CACHE:
        return _CACHE["nc"]
    nc = bacc.Bacc(
        "TRN2", target_bir_lowering=False, debug=False, num_devices=CORES
    )
    a = nc.dram_tensor("a_block", [R, N], F32, kind="ExternalInput").ap()
    out = nc.dram_tensor("out_block", [R, N], F32, kind="ExternalOutput").ap()

    with tile.TileContext(nc) as tc:
        with (
            tc.tile_pool(name="dram", bufs=1, space="DRAM") as dram,
            tc.tile_pool(name="res", bufs=1) as res,
            tc.tile_pool(name="stream", bufs=4) as stream,
            tc.tile_pool(name="cpool", bufs=1) as cpool,
            tc.tile_pool(name="small", bufs=1) as small,
        ):
            # separate DRAM tensors per collective half so AG1's input
            # dependency can never couple to stripes 4-7's writes
            isq_loc = [
                dram.tile([HAG], F32, name=f"isq_loc{g}") for g in range(2)
            ]
            isq_ag = [
                dram.tile(
                    [CORES * HAG], F32, addr_space="Shared", name=f"isq_ag{g}"
                )
                for g in range(2)
            ]

            part = small.tile([P, 4 * S], F32)   # partial row sums
            isq_sb = small.tile([P, S], F32)     # per-stripe row scale

            def finish_stripe(s, nparts):
                """Combine partials -> isq -> isq_sb + DRAM chunk."""
                for i in range(1, nparts):
                    nc.vector.tensor_add(
                        part[:, 4 * s : 4 * s + 1],
                        part[:, 4 * s : 4 * s + 1],
                        part[:, 4 * s + i : 4 * s + i + 1],
                    )
                nc.vector.reciprocal(
                    part[:, 4 * s : 4 * s + 1], part[:, 4 * s : 4 * s + 1]
                )
                nc.scalar.sqrt(
                    isq_sb[:, s : s + 1], part[:, 4 * s : 4 * s + 1]
                )
                g, off = divmod(s * P, HAG)
                nc.gpsimd.dma_start(
                    isq_loc[g][off : off + P].unsqueeze(1),
                    isq_sb[:, s : s + 1],
                )

            # ---- pass 1 ----
            # streamed stripes 0-3 first: their isq feeds AG1, and their
            # reduces free the stream slots for pass-2 reloads early
            nunit = 0
            for s in range(S - NRES):
                for q in range(N // QW):
                    t = stream.tile([P, QW], F32, tag="stream")
                    ld = nc.sync if nunit % 2 == 0 else nc.scalar
                    ld.dma_start(
                        t[:], a[s * P : (s + 1) * P, q * QW : (q + 1) * QW]
                    )
                    nc.vector.reduce_sum(
                        out=part[:, 4 * s + q : 4 * s + q + 1], in_=t[:], axis=X
                    )
                    nunit += 1
                finish_stripe(s, N // QW)

            ag_args = dict(
                replica_groups=[list(range(CORES))],
            )
            nc.gpsimd.collective_compute(
                "AllGather",
                mybir.AluOpType.bypass,
                ins=[isq_loc[0][:].opt()],
                outs=[isq_ag[0][:].opt()],
                **ag_args,
            )

            # resident stripes 4-7, kept for pass 2
            res_tiles = {}
            for s in range(S - NRES, S):
                for h in range(N // HW):
                    t = res.tile([P, HW], F32, tag=f"res{s}_{h}", bufs=1)
                    ld = nc.sync if nunit % 2 == 0 else nc.scalar
                    ld.dma_start(
                        t[:], a[s * P : (s + 1) * P, h * HW : (h + 1) * HW]
                    )
                    nc.vector.reduce_sum(
                        out=part[:, 4 * s + h : 4 * s + h + 1], in_=t[:], axis=X
                    )
                    res_tiles[(s, h)] = t
                    nunit += 1
                finish_stripe(s, N // HW)

            nc.gpsimd.collective_compute(
                "AllGather",
                mybir.AluOpType.bypass,
                ins=[isq_loc[1][:].opt()],
                outs=[isq_ag[1][:].opt()],
                **ag_args,
            )

            # column-scale broadcast. AG half g covers, within each 1024
            # column block, columns [g*512, g*512+512). isq_ag[g] is
            # ordered (core, stripe-offset): element k*512 + u = isq of
            # global row k*1024 + g*512 + u = scale for that column.
            # cb[g][h] holds half g's scales for output columns
            # [h*4096, (h+1)*4096), packed compactly ([m*512+u] layout):
            # one tile per AG half, so the early multiplies can never
            # pick up a false dependency on the late collective.
            cb = [
                [
                    cpool.tile(
                        [P, HW // 2],
                        F32,
                        tag=f"cb{g}{h}",
                        bufs=1,
                        name=f"cb{g}{h}",
                    )
                    for h in range(N // HW)
                ]
                for g in range(2)
            ]
            for g in range(2):
                for h in range(N // HW):
                    src = (
                        isq_ag[g][h * (HW // 2) : (h + 1) * (HW // 2)]
                        .rearrange("(m c) -> m c", c=HAG)
                        .unsqueeze(0)
                        .to_broadcast([P, HW // 1024, HAG])
                    )
                    nc.gpsimd.dma_start(
                        cb[g][h][:].rearrange("p (m c) -> p m c", c=HAG), src
                    )

            # ---- pass 2: out = (A * r) * c ----
            def scale_store(s, col0, t, width, g):
                """Scale + store the AG-half-g columns of tile t."""
                h, hoff = divmod(col0, HW)
                m0 = hoff // 1024
                m = width // 1024
                c_ap = cb[g][h][
                    :, m0 * HAG : (m0 + m) * HAG
                ].rearrange("p (m c) -> p m c", c=HAG)
                nc.vector.scalar_tensor_tensor(
                    out=c3(t[:], 0, width, g),
                    in0=c3(t[:], 0, width, g),
                    scalar=isq_sb[:, s : s + 1],
                    in1=c_ap,
                    op0=MUL,
                    op1=MUL,
                )
                nc.scalar.dma_start(
                    c3(out[s * P : (s + 1) * P, :], col0, width, g),
                    c3(t[:], 0, width, g),
                )

            # resident stripes: AG1-covered columns first (those flow
            # while AG2 is still waiting on the slowest core)
            for s in range(S - NRES, S):
                for h in range(N // HW):
                    scale_store(s, h * HW, res_tiles[(s, h)], HW, 0)
            for s in range(S - NRES, S):
                for h in range(N // HW):
                    scale_store(s, h * HW, res_tiles[(s, h)], HW, 1)

            # streamed stripes reload on the sync ring, quarter width
            for s in range(S - NRES):
                for q in range(N // QW):
                    t = stream.tile([P, QW], F32, tag="stream")
                    nc.sync.dma_start(
                        t[:], a[s * P : (s + 1) * P, q * QW : (q + 1) * QW]
                    )
                    scale_store(s, q * QW, t, QW, 0)
                    scale_store(s, q * QW, t, QW, 1)

    nc.compile()
    _CACHE["nc"] = nc
    return nc


def c3(ap, col0, width, g):
    """The AG-half-g columns of ap's column range [col0, col0+width):
    within each 1024-column block, columns [g*512, g*512+512), as a
    strided [P, width//1024, 512] access pattern."""
    return ap[:, col0 : col0 + width].rearrange("p (m c) -> p m c", c=1024)[
        :, :, g * HAG : (g + 1) * HAG
    ]


def kernel(adjacency_matrix):
    A = np.ascontiguousarray(np.asarray(adjacency_matrix, dtype=np.float32))
    assert A.shape == (N, N)
    nc = build_nc()
    in_maps = [
        {"a_block": np.ascontiguousarray(A[k * R : (k + 1) * R])}
        for k in range(CORES)
    ]
    res = run_bass_kernel_spmd(nc, in_maps, list(range(CORES)))
    return np.concatenate(
        [res.results[k]["out_block"] for k in range(CORES)], axis=0
    )



# revision 3
# speedup vs baseline: 1.1008x; 1.1008x over previous
"""Laplacian normalization kernel for Trainium2 (8 NeuronCores, SPMD).

out = D^-1/2 A D^-1/2 where D = diag(row sums of A), A: [8192, 8192] fp32.

Sharding: rows split across 8 cores (1024 rows each, 8 stripes of 128).

Single-load design (64MB HBM traffic/core vs 80MB for load-twice):
  pass 1: stream each stripe once as f32 chunks on alternating HWDGE
    rings. One scalar-engine ACT op per chunk does BOTH jobs: casts the
    chunk into a resident bf16 SBUF tile (16MB total — fits) and emits
    the per-partition row sum via accum_out. The vector engine stays
    empty in pass 1 so the tiny finish chains (combine/reciprocal) never
    queue behind bulk work.
  TWO AllGathers of the isq vector halves (stripes 0-3 after ~half the
    loads, stripes 4-7 at the end) so the first AG's latency hides under
    the remaining loads and the second's under the first stores.
  pass 2: out = (res_bf16 * r[:,None]) * c[None,:] on the vector engine
    into f32 staging tiles, stored on alternating rings. No HBM re-read.

isq chunk writes ride the sync HWDGE ring, batched after the next
stripe's loads are already enqueued, so their sqrt-chain latency never
stalls a load sitting behind them in ring FIFO order. The gpsimd queue
holds ONLY [AG1, cb0, AG2, cb1] so a collective trigger can never queue
behind unrelated DMAs (the failure mode of the previous version).

bf16 residents cost ~0.2% relative error on A; tolerance is 2e-2.
"""

import sys

sys.path.insert(0, "/opt/trn_rl_repo")

import numpy as np

import concourse.bacc as bacc
import concourse.tile as tile
from concourse import mybir
from concourse.bass_utils import run_bass_kernel_spmd

N = 8192          # full matrix dim
CORES = 8
R = N // CORES    # rows per core: 1024
P = 128           # partitions
S = R // P        # row stripes per core: 8
QW = 2048         # load chunk width
NQ = N // QW      # chunks per stripe: 4
HW = 4096         # pass-2 column block width
HAG = R // 2      # isq elements per collective half: 512
F32 = mybir.dt.float32
BF16 = mybir.dt.bfloat16
MUL = mybir.AluOpType.mult
X = mybir.AxisListType.X
COPY = mybir.ActivationFunctionType.Copy

_CACHE = {}


def build_nc():
    if "nc" in _CACHE:
        return _CACHE["nc"]
    nc = bacc.Bacc(
        "TRN2", target_bir_lowering=False, debug=False, num_devices=CORES
    )
    a = nc.dram_tensor("a_block", [R, N], F32, kind="ExternalInput").ap()
    out = nc.dram_tensor("out_block", [R, N], F32, kind="ExternalOutput").ap()

    with tile.TileContext(nc) as tc:
        with (
            tc.tile_pool(name="dram", bufs=1, space="DRAM") as dram,
            tc.tile_pool(name="res", bufs=1) as res,
            tc.tile_pool(name="io", bufs=4) as io,
            tc.tile_pool(name="cpool", bufs=1) as cpool,
            tc.tile_pool(name="small", bufs=1) as small,
        ):
            isq_loc = [
                dram.tile([HAG], F32, name=f"isq_loc{g}") for g in range(2)
            ]
            isq_ag = [
                dram.tile(
                    [CORES * HAG], F32, addr_space="Shared", name=f"isq_ag{g}"
                )
                for g in range(2)
            ]

            part = small.tile([P, S * NQ], F32)   # per-chunk row sums
            deg = small.tile([P, S], F32)         # combined per-stripe sums
            isq_sb = small.tile([P, S], F32)      # 1/sqrt(deg)

            res_t = [
                res.tile([P, N], BF16, tag=f"res{s}", bufs=1, name=f"res{s}")
                for s in range(S)
            ]

            def finish_stripe(s):
                """partials -> deg -> isq_sb[:, s] (vector+scalar, tiny)."""
                nc.vector.reduce_sum(
                    out=deg[:, s : s + 1],
                    in_=part[:, s * NQ : s * NQ + NQ],
                    axis=X,
                )
                nc.vector.reciprocal(deg[:, s : s + 1], deg[:, s : s + 1])
                nc.scalar.sqrt(isq_sb[:, s : s + 1], deg[:, s : s + 1])

            def write_isq(g):
                """isq chunks for stripes 4g..4g+3 -> isq_loc[g] (sync ring)."""
                for i in range(4):
                    s = 4 * g + i
                    nc.sync.dma_start(
                        isq_loc[g][i * P : (i + 1) * P].unsqueeze(1),
                        isq_sb[:, s : s + 1],
                    )

            # ---- pass 1: load once, cast+reduce per chunk ----
            nunit = 0
            for s in range(S):
                for q in range(NQ):
                    t = io.tile([P, QW], F32, tag="io")
                    ld = nc.sync if nunit % 2 == 0 else nc.scalar
                    ld.dma_start(
                        t[:], a[s * P : (s + 1) * P, q * QW : (q + 1) * QW]
                    )
                    nc.scalar.activation(
                        res_t[s][:, q * QW : (q + 1) * QW],
                        t[:],
                        COPY,
                        accum_out=part[:, s * NQ + q : s * NQ + q + 1],
                    )
                    nunit += 1
                finish_stripe(s)
                if s == 4:
                    # stripes 0-3's isq writes, enqueued behind stripe 4's
                    # loads so their latency can't stall a load in FIFO
                    write_isq(0)
            write_isq(1)

            ag_args = dict(replica_groups=[list(range(CORES))])

            # cb[g][h]: AG half g's column scales for output columns
            # [h*4096, (h+1)*4096), packed [P, 4*512] (bf16), replicated
            # across partitions.
            cb = [
                [
                    cpool.tile(
                        [P, HW // 2], BF16, tag=f"cb{g}{h}", bufs=1,
                        name=f"cb{g}{h}",
                    )
                    for h in range(N // HW)
                ]
                for g in range(2)
            ]

            def bcast_cb(g):
                for h in range(N // HW):
                    src = (
                        isq_ag[g][h * (HW // 2) : (h + 1) * (HW // 2)]
                        .rearrange("(m c) -> m c", c=HAG)
                        .unsqueeze(0)
                        .to_broadcast([P, HW // 1024, HAG])
                    )
                    nc.gpsimd.dma_start(
                        cb[g][h][:].rearrange("p (m c) -> p m c", c=HAG), src
                    )

            # gpsimd queue: [AG1, cb0, AG2, cb1] and nothing else
            nc.gpsimd.collective_compute(
                "AllGather", mybir.AluOpType.bypass,
                ins=[isq_loc[0][:].opt()], outs=[isq_ag[0][:].opt()],
                **ag_args,
            )
            bcast_cb(0)
            nc.gpsimd.collective_compute(
                "AllGather", mybir.AluOpType.bypass,
                ins=[isq_loc[1][:].opt()], outs=[isq_ag[1][:].opt()],
                **ag_args,
            )
            bcast_cb(1)

            # ---- pass 2: out = (res * r) * c, AG1-covered columns first ----
            def c3(ap, h, g):
                """AG-half-g columns of ap's 4096-col block h as
                [P, 4, 512]: within each 1024-col block, cols
                [g*512, (g+1)*512)."""
                return ap[:, h * HW : (h + 1) * HW].rearrange(
                    "p (m c) -> p m c", c=1024
                )[:, :, g * HAG : (g + 1) * HAG]

            nunit = 0
            for g in range(2):
                for s in range(S):
                    for h in range(N // HW):
                        st = io.tile([P, QW], F32, tag="io")
                        stv = st[:].rearrange("p (m c) -> p m c", c=HAG)
                        nc.vector.scalar_tensor_tensor(
                            out=stv,
                            in0=c3(res_t[s], h, g),
                            scalar=isq_sb[:, s : s + 1],
                            in1=cb[g][h][:].rearrange(
                                "p (m c) -> p m c", c=HAG
                            ),
                            op0=MUL,
                            op1=MUL,
                        )
                        std = nc.sync if nunit % 2 == 0 else nc.scalar
                        std.dma_start(
                            c3(out[s * P : (s + 1) * P, :], h, g), stv
                        )
                        nunit += 1

    nc.compile()
    _CACHE["nc"] = nc
    return nc


def kernel(adjacency_matrix):
    A = np.ascontiguousarray(np.asarray(adjacency_matrix, dtype=np.float32))
    assert A.shape == (N, N)
    nc = build_nc()
    in_maps = [
        {"a_block": np.ascontiguousarray(A[k * R : (k + 1) * R])}
        for k in range(CORES)
    ]
    res = run_bass_kernel_spmd(nc, in_maps, list(range(CORES)))
    return np.concatenate(
        [res.results[k]["out_block"] for k in range(CORES)], axis=0
    )


# revision 9
# speedup vs baseline: 1.2672x; 1.1511x over previous
"""Laplacian normalization kernel for Trainium2 (8 NeuronCores, SPMD).

out = D^-1/2 A D^-1/2 where D = diag(row sums of A), A: [8192, 8192] fp32.

Sharding: rows split across 8 cores (1024 rows each, 8 stripes of 128).

Single-load design (64MB HBM traffic/core vs 80MB for load-twice):
  pass 1: stream each stripe once as f32 chunks on alternating HWDGE
    rings. One scalar-engine ACT op per chunk does BOTH jobs: casts the
    chunk into a resident bf16 SBUF tile (16MB total — fits) and emits
    the per-partition row sum via accum_out. Loads are dispatched SIX
    chunks ahead of their casts in the scalar queue: a cast waiting on
    its DMA otherwise head-of-line-blocks the next load dispatch behind
    it (measured: collapses load BW from ~330 to ~160GB/s with the
    naive load/cast/load/cast interleave).
  TWO AllGathers of the isq vector halves (stripes 0-3 after ~half the
    loads, stripes 4-7 at the end) so the first AG's latency hides under
    the remaining loads and the second's under the first stores.
  pass 2: out = (res_bf16 * r[:,None]) * c[None,:] on the vector engine
    into f32 staging tiles, stored on alternating rings. No HBM re-read.

isq chunk writes ride the sync HWDGE ring, batched after the next
stripe's loads are already enqueued, so their sqrt-chain latency never
stalls a load sitting behind them in ring FIFO order. The gpsimd queue
holds ONLY [AG1, cb0, AG2, cb1] so a collective trigger can never queue
behind unrelated DMAs (the failure mode of the previous version).

bf16 residents cost ~0.2% relative error on A; tolerance is 2e-2.
"""

import sys

sys.path.insert(0, "/opt/trn_rl_repo")

import numpy as np

import concourse.bacc as bacc
import concourse.tile as tile
from concourse import mybir
from concourse.bass_utils import run_bass_kernel_spmd

N = 8192          # full matrix dim
CORES = 8
R = N // CORES    # rows per core: 1024
P = 128           # partitions
S = R // P        # row stripes per core: 8
QW = 2048         # load chunk width
NQ = N // QW      # chunks per stripe: 4
HW = 4096         # pass-2 column block width
HAG = R // 2      # isq elements per collective half: 512
F32 = mybir.dt.float32
BF16 = mybir.dt.bfloat16
MUL = mybir.AluOpType.mult
X = mybir.AxisListType.X
COPY = mybir.ActivationFunctionType.Copy

_CACHE = {}


def build_nc():
    if "nc" in _CACHE:
        return _CACHE["nc"]
    nc = bacc.Bacc(
        "TRN2", target_bir_lowering=False, debug=False, num_devices=CORES
    )
    a = nc.dram_tensor("a_block", [R, N], F32, kind="ExternalInput").ap()
    out = nc.dram_tensor("out_block", [R, N], F32, kind="ExternalOutput").ap()

    with tile.TileContext(nc) as tc:
        with (
            tc.tile_pool(name="dram", bufs=1, space="DRAM") as dram,
            tc.tile_pool(name="res", bufs=1) as res,
            tc.tile_pool(name="io", bufs=6) as io,
            tc.tile_pool(name="cpool", bufs=1) as cpool,
            tc.tile_pool(name="small", bufs=1) as small,
        ):
            isq_loc = [
                dram.tile([HAG], F32, name=f"isq_loc{g}") for g in range(2)
            ]
            isq_ag = [
                dram.tile(
                    [CORES * HAG], F32, addr_space="Shared", name=f"isq_ag{g}"
                )
                for g in range(2)
            ]

            part = small.tile([P, S * NQ], F32)   # per-chunk row sums
            deg = small.tile([P, S], F32)         # combined per-stripe sums
            isq_sb = small.tile([P, S], F32)      # 1/sqrt(deg)

            res_t = [
                res.tile([P, N], BF16, tag=f"res{s}", bufs=1, name=f"res{s}")
                for s in range(S)
            ]

            def finish_stripe(s):
                """partials -> deg -> 1/deg (vector, tiny, inline)."""
                nc.vector.reduce_sum(
                    out=deg[:, s : s + 1],
                    in_=part[:, s * NQ : s * NQ + NQ],
                    axis=X,
                )
                nc.vector.reciprocal(deg[:, s : s + 1], deg[:, s : s + 1])

            def sqrt_and_write_isq(g):
                """sqrt + isq chunk writes for stripes 4g..4g+3, batched at
                a point where their recip deps are long done, so neither
                the sqrts (scalar queue) nor the tiny writes (sync ring)
                ever stall a load dispatch sitting behind them."""
                for i in range(4):
                    s = 4 * g + i
                    nc.scalar.sqrt(isq_sb[:, s : s + 1], deg[:, s : s + 1])
                for i in range(4):
                    s = 4 * g + i
                    nc.sync.dma_start(
                        isq_loc[g][i * P : (i + 1) * P].unsqueeze(1),
                        isq_sb[:, s : s + 1],
                    )

            # ---- pass 1: load once, cast+reduce per chunk ----
            NU = S * NQ          # 32 chunk units
            LOOK = 6             # dispatch-ahead depth = io pool size
            tiles = [None] * NU

            def dispatch(u):
                s, q = divmod(u, NQ)
                t = io.tile([P, QW], F32, tag="io", name=f"io{u % LOOK}")
                tiles[u] = t
                ld = nc.sync if u % 2 == 0 else nc.scalar
                ld.dma_start(
                    t[:], a[s * P : (s + 1) * P, q * QW : (q + 1) * QW]
                )

            for u in range(LOOK):
                dispatch(u)
            for u in range(NU):
                if u + LOOK < NU:
                    dispatch(u + LOOK)
                s, q = divmod(u, NQ)
                nc.scalar.activation(
                    res_t[s][:, q * QW : (q + 1) * QW],
                    tiles[u][:],
                    COPY,
                    accum_out=part[:, u : u + 1],
                )
                if q == NQ - 1:
                    finish_stripe(s)
                if u == 23:
                    sqrt_and_write_isq(0)
            sqrt_and_write_isq(1)

            ag_args = dict(replica_groups=[list(range(CORES))])

            # cb[g][h]: AG half g's column scales for output columns
            # [h*4096, (h+1)*4096), packed [P, 4*512] (bf16), replicated
            # across partitions.
            cb = [
                [
                    cpool.tile(
                        [P, HW // 2], BF16, tag=f"cb{g}{h}", bufs=1,
                        name=f"cb{g}{h}",
                    )
                    for h in range(N // HW)
                ]
                for g in range(2)
            ]

            def bcast_cb(g):
                for h in range(N // HW):
                    src = (
                        isq_ag[g][h * (HW // 2) : (h + 1) * (HW // 2)]
                        .rearrange("(m c) -> m c", c=HAG)
                        .unsqueeze(0)
                        .to_broadcast([P, HW // 1024, HAG])
                    )
                    nc.gpsimd.dma_start(
                        cb[g][h][:].rearrange("p (m c) -> p m c", c=HAG), src
                    )

            # gpsimd queue: [AG1, cb0, AG2, cb1] and nothing else
            nc.gpsimd.collective_compute(
                "AllGather", mybir.AluOpType.bypass,
                ins=[isq_loc[0][:].opt()], outs=[isq_ag[0][:].opt()],
                **ag_args,
            )
            bcast_cb(0)
            nc.gpsimd.collective_compute(
                "AllGather", mybir.AluOpType.bypass,
                ins=[isq_loc[1][:].opt()], outs=[isq_ag[1][:].opt()],
                **ag_args,
            )
            bcast_cb(1)

            # ---- pass 2: out = (res * r) * c, AG1-covered columns first ----
            def c3(ap, h, g):
                """AG-half-g columns of ap's 4096-col block h as
                [P, 4, 512]: within each 1024-col block, cols
                [g*512, (g+1)*512)."""
                return ap[:, h * HW : (h + 1) * HW].rearrange(
                    "p (m c) -> p m c", c=1024
                )[:, :, g * HAG : (g + 1) * HAG]

            nunit = 0
            for g in range(2):
                for s in range(S):
                    for h in range(N // HW):
                        st = io.tile([P, QW], F32, tag="io")
                        stv = st[:].rearrange("p (m c) -> p m c", c=HAG)
                        nc.vector.scalar_tensor_tensor(
                            out=stv,
                            in0=c3(res_t[s], h, g),
                            scalar=isq_sb[:, s : s + 1],
                            in1=cb[g][h][:].rearrange(
                                "p (m c) -> p m c", c=HAG
                            ),
                            op0=MUL,
                            op1=MUL,
                        )
                        std = nc.sync if nunit % 2 == 0 else nc.scalar
                        std.dma_start(
                            c3(out[s * P : (s + 1) * P, :], h, g), stv
                        )
                        nunit += 1

    nc.compile()
    _CACHE["nc"] = nc
    return nc


def kernel(adjacency_matrix):
    A = np.ascontiguousarray(np.asarray(adjacency_matrix, dtype=np.float32))
    assert A.shape == (N, N)
    nc = build_nc()
    in_maps = [
        {"a_block": np.ascontiguousarray(A[k * R : (k + 1) * R])}
        for k in range(CORES)
    ]
    res = run_bass_kernel_spmd(nc, in_maps, list(range(CORES)))
    return np.concatenate(
        [res.results[k]["out_block"] for k in range(CORES)], axis=0
    )
